# revision 8
# baseline (speedup 1.0000x reference)
"""Causal self-attention Bass/Tile kernel for Trainium2, 8-core data-parallel.

Problem: B=8, T=1024, C=1024, H=16, D=64, fp32.
  qkv = x @ w_attn + b_attn; causal SDPA over 16 heads; out = y @ w_proj + b_proj

Sharding: batch (B=8) across the 8 NeuronCores — one batch element per core,
no collectives. Each core computes its full [T, C] output slice.
"""

import sys
from contextlib import ExitStack

import numpy as np

import concourse.bass as bass
import concourse.tile as tile
from concourse import mybir
from concourse.bass_utils import run_bass_kernel_spmd
from concourse.masks import make_identity

F32 = mybir.dt.float32
AF = mybir.ActivationFunctionType

# ---------------------------------------------------------------------------
# Workaround: this walrus build rejects instructions carrying more than one
# sem wait ("Too many sync wait commands").  Post-pass: move excess waits
# onto fresh single-wait NoOps inserted just before the instruction in its
# engine stream.
# ---------------------------------------------------------------------------
_MAX_WAITS = 1


def _split_sync_waits(nc, max_waits=_MAX_WAITS):
    uid = 0
    for f in nc.m.functions:
        for blk in f.blocks:
            insts = blk.instructions
            i = 0
            while i < len(insts):
                inst = insts[i]
                si = inst.sync_info
                if si is not None and len(si.on_wait) > max_waits:
                    waits = list(si.on_wait)
                    keep = waits[-max_waits:]
                    extra = waits[:-max_waits]
                    inst.sync_info = mybir.SyncInfo(
                        on_wait=keep, on_update=list(si.on_update)
                    )
                    pos = i
                    for j in range(0, len(extra), max_waits):
                        nop = mybir.InstNoOp(
                            name=f"wsplit-{uid}",
                            engine=inst.engine,
                            ins=[],
                            outs=[],
                            sync_info=mybir.SyncInfo(
                                on_wait=extra[j : j + max_waits], on_update=[]
                            ),
                        )
                        uid += 1
                        insts.insert(pos, nop)
                        pos += 1
                        i += 1
                i += 1


# ---------------------------------------------------------------------------
# Kernel build
# ---------------------------------------------------------------------------
N_CORES = 8
T = 1024
C = 1024
H = 16
D = C // H  # 64
C3 = 3 * C
P = 128  # partitions
NT = T // P      # 8 t-chunks
NCH = C // P     # 8 c-chunks
NQK = 2 * C // P  # 16 m-chunks covering q and k rows of qkv^T
TQG = 512        # tq group width (fp32 matmul max N)
NG = T // TQG    # 2 tq groups
HPAIRS = H // 2  # 8 head pairs; pair hp = heads 2hp (parts 0-63), 2hp+1 (64-127)
SCALE = 1.0 / np.sqrt(D)


def _emit_kernel(nc, tc, ctx, x_d, wa_d, ba_d, wp_d, bp_d, out_d):
    const = ctx.enter_context(tc.tile_pool(name="const", bufs=1))
    persist = ctx.enter_context(tc.tile_pool(name="persist", bufs=1))

    # --- constants -------------------------------------------------------
    ident = const.tile([P, P], F32)
    make_identity(nc, ident)

    # trimask[p, f] = 1.0 where f >= p else 0.0   (S^T diag block: keep tq>=tk)
    trimask = const.tile([P, P], F32)
    nc.gpsimd.memset(trimask, 1.0)
    nc.gpsimd.affine_select(
        out=trimask, in_=trimask, compare_op=mybir.AluOpType.is_ge, fill=0.0,
        base=0, pattern=[[1, P]], channel_multiplier=-1,
    )

    ones_sb = const.tile([P, P], F32)
    nc.vector.memset(ones_sb, 1.0)

    # b_attn as [128, 24] (partition p of column m = bias[m*128+p])
    ba_sb = const.tile([P, C3 // P], F32)
    nc.sync.dma_start(out=ba_sb, in_=ba_d.rearrange("(m p) -> p m", p=P))
    # v-bias and proj-bias as single rows
    bv_row = const.tile([1, C], F32)
    nc.sync.dma_start(out=bv_row, in_=ba_d[2 * C : 3 * C].rearrange("(o c) -> o c", o=1))
    bp_row = const.tile([1, C], F32)
    nc.sync.dma_start(out=bp_row, in_=bp_d.rearrange("(o c) -> o c", o=1))

    # --- phase 1: xT[c_chunk][p=c, t] ------------------------------------
    xT = persist.tile([P, NCH, T], F32, tag="big", name="xT")
    with tc.tile_pool(name="xnat", bufs=2) as xnat, \
         tc.tile_pool(name="tp_ps", bufs=4, space="PSUM") as tp_ps:
        for tch in range(NT):
            xn = xnat.tile([P, C], F32)
            nc.sync.dma_start(out=xn, in_=x_d[tch * P : (tch + 1) * P, :])
            for cch in range(NCH):
                ps = tp_ps.tile([P, P], F32)
                nc.tensor.transpose(ps, xn[:, cch * P : (cch + 1) * P], ident)
                nc.vector.tensor_copy(xT[:, cch, tch * P : (tch + 1) * P], ps)

    # --- phase 2: qkT[m_chunk][p=row of qkv^T, t] for m in 0..15 ---------
    qkT = persist.tile([P, NQK, T], F32)
    with tc.tile_pool(name="wa", bufs=18) as wa_pool, \
         tc.tile_pool(name="qk_ps", bufs=4, space="PSUM") as qk_ps:
        for m in range(NQK):
            wts = []
            for k in range(NCH):
                wt = wa_pool.tile([P, P], F32, tag="wa")
                nc.sync.dma_start(
                    out=wt, in_=wa_d[k * P : (k + 1) * P, m * P : (m + 1) * P]
                )
                wts.append(wt)
            pss = [qk_ps.tile([P, TQG], F32, tag="qkps", name=f"qkps_{m}_{g}") for g in range(NG)]
            for k in range(NCH):
                for g in range(NG):
                    nc.tensor.matmul(
                        pss[g], lhsT=wts[k], rhs=xT[:, k, g * TQG : (g + 1) * TQG],
                        start=(k == 0), stop=(k == NCH - 1),
                    )
            for g in range(NG):
                nc.vector.tensor_scalar_add(
                    qkT[:, m, g * TQG : (g + 1) * TQG], pss[g], ba_sb[:, m : m + 1]
                )

    # --- phase 3: v natural, written as vaug[p=t, tk_chunk, head, 65] ----
    # column 64 of each head block is 1.0 (fused row-sum for softmax denom)
    vaug = persist.tile([P, NT, H, D + 1], F32)
    nc.vector.memset(vaug[:, :, :, D : D + 1], 1.0)
    with tc.tile_pool(name="wv", bufs=4) as wv_pool, \
         tc.tile_pool(name="v_ps", bufs=8, space="PSUM") as v_ps:
        for n in range(C // TQG):  # 2 halves of v columns -> heads n*8..n*8+7
            pss = [v_ps.tile([P, TQG], F32, tag="vps", name=f"vps_{n}_{t}") for t in range(NT)]
            for k in range(NCH):
                wv = wv_pool.tile([P, TQG], F32, tag="wv")
                nc.sync.dma_start(
                    out=wv,
                    in_=wa_d[k * P : (k + 1) * P, 2 * C + n * TQG : 2 * C + (n + 1) * TQG],
                )
                for tch in range(NT):
                    nc.tensor.matmul(
                        pss[tch], lhsT=xT[:, k, tch * P : (tch + 1) * P], rhs=wv,
                        start=(k == 0), stop=False,
                    )
            for tch in range(NT):
                nc.tensor.matmul(
                    pss[tch], lhsT=ones_sb[0:1, 0:P],
                    rhs=bv_row[0:1, n * TQG : (n + 1) * TQG],
                    start=False, stop=True,
                )
                nc.vector.tensor_copy(
                    vaug[:, tch, n * 8 : (n + 1) * 8, 0:D],
                    pss[tch].rearrange("p (h d) -> p h d", h=8),
                )

    # --- phase 4: attention per head pair --------------------------------
    # yT reuses xT's slot (same tag; Tile inserts the WAR dependency)
    yT = persist.tile([P, NCH, T], F32, tag="big", name="yT")
    attn_ctx = ExitStack()
    attn_exp = attn_ctx.enter_context(tc.tile_pool(name="attn_exp", bufs=6))
    s_ps = attn_ctx.enter_context(tc.tile_pool(name="s_ps", bufs=4, space="PSUM"))
    y_ps = attn_ctx.enter_context(tc.tile_pool(name="y_ps", bufs=2, space="PSUM"))
    norm_sb = attn_ctx.enter_context(tc.tile_pool(name="norm_sb", bufs=2))

    for hp in range(HPAIRS):
        hA, hB = 2 * hp, 2 * hp + 1
        for g in range(NG):
            yps = {
                "A": y_ps.tile([D + 1, TQG], F32, tag="yps", name=f"yps_a_{hp}_{g}"),
                "B": y_ps.tile([D + 1, TQG], F32, tag="yps", name=f"yps_b_{hp}_{g}"),
            }
            n_i = 4 * g + 4
            for i in range(n_i):
                j = i - 4 * g
                off = P * j if j >= 0 else 0
                N = TQG - off
                tq_lo = g * TQG + off
                tq_hi = (g + 1) * TQG
                for head, lo in (("A", 0), ("B", 64)):
                    h = hA if head == "A" else hB
                    sp = s_ps.tile([P, TQG], F32, tag="sps")
                    nc.tensor.matmul(
                        sp[:, 0:N],
                        lhsT=qkT[lo : lo + 64, NCH + hp, i * P : (i + 1) * P],
                        rhs=qkT[lo : lo + 64, hp, tq_lo:tq_hi],
                        tile_position=(lo, 0),
                    )
                    e = attn_exp.tile([P, TQG], F32, tag="exp")
                    nc.scalar.activation(e[:, 0:N], sp[:, 0:N], AF.Exp, scale=float(SCALE))
                    if j >= 0:
                        nc.vector.tensor_mul(e[:, 0:P], e[:, 0:P], trimask)
                    nc.tensor.matmul(
                        yps[head][:, off:TQG], lhsT=vaug[:, i, h, :], rhs=e[:, 0:N],
                        start=(i == 0), stop=(i == n_i - 1),
                    )
            # normalize: y[d, tq] / sum[tq]
            for head, lo in (("A", 0), ("B", 64)):
                yp = yps[head]
                rs = norm_sb.tile([D + 1, TQG], F32, tag="rs")
                nc.vector.reciprocal(rs[D : D + 1, :], yp[D : D + 1, :])
                # broadcast recip row (partition 64) to partitions 0..63 via PE
                bc = s_ps.tile([P, TQG], F32, tag="sps")
                nc.tensor.matmul(
                    bc[0:D, :], lhsT=ones_sb[D : D + 1, 0:D], rhs=rs[D : D + 1, :],
                    tile_position=(64, 0),
                )
                rb = norm_sb.tile([D, TQG], F32, tag="rb")
                nc.vector.tensor_copy(rb, bc[0:D, :])
                if head == "A":
                    nc.vector.tensor_mul(
                        yT[0:D, hp, g * TQG : (g + 1) * TQG], yp[0:D, :], rb
                    )
                else:
                    stg = norm_sb.tile([D, TQG], F32, tag="stg")
                    nc.vector.tensor_mul(stg, yp[0:D, :], rb)
                    nc.sync.dma_start(
                        out=yT[64:128, hp, g * TQG : (g + 1) * TQG], in_=stg
                    )

    # --- phase 5: out = yT^T-contract @ w_proj + b_proj ------------------
    attn_ctx.close()
    wproj_pool = ctx.enter_context(tc.tile_pool(name="wproj_pool", bufs=1))
    wproj_sb = wproj_pool.tile([P, NCH, C], F32)
    for k in range(NCH):
        nc.sync.dma_start(out=wproj_sb[:, k, :], in_=wp_d[k * P : (k + 1) * P, :])
    proj_ps = ctx.enter_context(tc.tile_pool(name="proj_ps", bufs=4, space="PSUM"))
    out_sb = ctx.enter_context(tc.tile_pool(name="out_sb", bufs=3))
    for m in range(NT):
        pss = [proj_ps.tile([P, TQG], F32, tag="pps", name=f"pps_{m}_{n}") for n in range(C // TQG)]
        for k in range(NCH):
            for n in range(C // TQG):
                nc.tensor.matmul(
                    pss[n], lhsT=yT[:, k, m * P : (m + 1) * P],
                    rhs=wproj_sb[:, k, n * TQG : (n + 1) * TQG],
                    start=(k == 0), stop=False,
                )
        ob = out_sb.tile([P, C], F32, tag="ob")
        for n in range(C // TQG):
            nc.tensor.matmul(
                pss[n], lhsT=ones_sb[0:1, 0:P],
                rhs=bp_row[0:1, n * TQG : (n + 1) * TQG],
                start=False, stop=True,
            )
            nc.vector.tensor_copy(ob[:, n * TQG : (n + 1) * TQG], pss[n])
        nc.sync.dma_start(out=out_d[m * P : (m + 1) * P, :], in_=ob)


def build_nc(n_cores=N_CORES):
    nc = bass.Bass("TRN2", target_bir_lowering=False, debug=False, num_devices=n_cores)
    x_d = nc.dram_tensor("x", [T, C], F32, kind="ExternalInput").ap()
    wa_d = nc.dram_tensor("w_attn", [C, C3], F32, kind="ExternalInput").ap()
    ba_d = nc.dram_tensor("b_attn", [C3], F32, kind="ExternalInput").ap()
    wp_d = nc.dram_tensor("w_proj", [C, C], F32, kind="ExternalInput").ap()
    bp_d = nc.dram_tensor("b_proj", [C], F32, kind="ExternalInput").ap()
    out_d = nc.dram_tensor("out", [T, C], F32, kind="ExternalOutput").ap()
    with tile.TileContext(nc) as tc:
        with ExitStack() as ctx:
            _emit_kernel(nc, tc, ctx, x_d, wa_d, ba_d, wp_d, bp_d, out_d)
    _split_sync_waits(nc)
    return nc


_NC_CACHE = {}


def _get_nc(n_cores=N_CORES):
    if n_cores not in _NC_CACHE:
        _NC_CACHE[n_cores] = build_nc(n_cores)
    return _NC_CACHE[n_cores]


def kernel(x, attn_mask, w_attn, b_attn, w_proj, b_proj):
    """Full inputs in, full output out. attn_mask is causal (hardcoded)."""
    x = np.ascontiguousarray(np.asarray(x, dtype=np.float32))
    w_attn = np.ascontiguousarray(np.asarray(w_attn, dtype=np.float32))
    b_attn = np.ascontiguousarray(np.asarray(b_attn, dtype=np.float32))
    w_proj = np.ascontiguousarray(np.asarray(w_proj, dtype=np.float32))
    b_proj = np.ascontiguousarray(np.asarray(b_proj, dtype=np.float32))
    B = x.shape[0]
    assert B == N_CORES and x.shape == (B, T, C)

    nc = _get_nc(N_CORES)
    in_maps = [
        {"x": x[b], "w_attn": w_attn, "b_attn": b_attn,
         "w_proj": w_proj, "b_proj": b_proj}
        for b in range(B)
    ]
    res = run_bass_kernel_spmd(nc, in_maps, core_ids=list(range(N_CORES)))
    return np.stack([res.results[b]["out"] for b in range(B)], axis=0)
